# revision 20
# baseline (speedup 1.0000x reference)
"""BioSignalEmbed Trainium2 Bass kernel.

Contract: kernel(**inputs) -> np.ndarray. Full inputs in, full output out.
Sharding: pure data parallel - one batch element per NeuronCore (8 cores).

Math decomposition (per core, x = signal[b] of shape (65536, 64)):
  Phase A (TensorE): x viewed as [128 samples, 512 q-cols, 64 ch]. For each
    pair of q-cols i, a stationary tile X[:, (q2,c)] (M=128) is multiplied by
    a moving DFT matrix D [128, 320]:
      cols (w, f), w=0..3, f=0..63: scaled 64-pt rfft cos/sin projections of
      window t = 4q+w (w=3 is the head half; the tail comes from a second
      accumulating matmul with the q-shifted stationary and D2 = cols 256:320).
    Scaling: cos_0, cos_32 rows x 1/8; others x sqrt(2)/8, which makes
    sum_f F_f^2 = the window sum of squares exactly (Parseval), so
      var*63 = sum_{f>=1} F_f^2.
  Phase B (ScalarE/VectorE): squares with fused Parseval accumulation,
    re^2+im^2 pair adds, sqrt -> mag'_k (k=1..24), F0 (mean*8), std_raw.
  B->C: one clean SBUF->SBUF DMA per (feat-pair, q-parity) transposes
    [(q2,c) partitions, feat free] -> [(f,c) partitions, t free].
  Phase C (TensorE): z[t,:] = sum over 13 K-tiles of MT_kt.T @ HmT_kt, where
    Hm folds band means, per-channel 7->8 projection, chan_b, mix_w, and all
    scale corrections. PSUM drained with fused +table (sinusoidal PE + bias).
Output rows t=0..2046 -> out[b, 1:2048]; out[b, 0] = marker (host).
"""

import numpy as np

import concourse.bass as bass
import concourse.bacc as bacc
import concourse.mybir as mybir
import concourse.tile as tile
import jax
from jax.sharding import Mesh, NamedSharding, PartitionSpec
from jax.experimental.shard_map import shard_map
from concourse import bass2jax

F32 = mybir.dt.float32
F32R = mybir.dt.float32r
AF = mybir.ActivationFunctionType

B, T, C = 8, 65536, 64
Q = T // 128            # 512 q-cols of 128 samples
NSC = 8                 # superchunks
QSC = Q // NSC          # 64 q-cols per superchunk
NST = QSC // 2          # 32 stationary pairs per superchunk
NT = 2047               # valid windows
HID = 512
NKT = 13                # phase-C contraction tiles (26 feats x 64 ch / 128)

LAST_RESULTS = None     # test harness can read profiling info from here


# ----------------------------------------------------------------- host math
def _build_d():
    n = np.arange(64)
    c64 = np.zeros((64, 64), np.float64)
    for f in range(33):
        s = 1 / 8 if f in (0, 32) else np.sqrt(2) / 8
        c64[:, f] = s * np.cos(2 * np.pi * f * n / 64)
    for f in range(33, 64):
        c64[:, f] = (np.sqrt(2) / 8) * np.sin(2 * np.pi * (f - 32) * n / 64)
    d = np.zeros((128, 320), np.float64)
    for w in range(3):
        d[32 * w:32 * w + 64, 64 * w:64 * w + 64] = c64
    d[96:128, 192:256] = c64[0:32]       # w3 head
    d[0:32, 256:320] = c64[32:64]        # w3 tail (shifted stationary)
    return d.astype(np.float32)


def _build_hmt(chan_w, mix_w):
    """HmT (1664, 512): row f*64+c consumes m26 feat f of channel c.
    m26 = [mag'_1..24, F0, std_raw]."""
    s = np.sqrt(2) / 8
    bmat = np.zeros((26, 7), np.float64)   # m26 -> feats7
    bmat[0, 1] = 1 / s
    bmat[1, 2] = 0.5 / s
    bmat[2, 2] = 0.5 / s
    for k in range(4, 8):
        bmat[k - 1, 3] = 0.25 / s
    for k in range(8, 25):
        bmat[k - 1, 4] = (1 / 17) / s
    bmat[24, 5] = 1 / 8                    # mean = F0/8
    bmat[25, 6] = 1 / np.sqrt(63)          # std = std_raw/sqrt(63)
    hmt = np.zeros((26 * 64, HID), np.float64)
    cw = chan_w.astype(np.float64)
    mw = mix_w.astype(np.float64)
    for c in range(C):
        g = mw[:, c * 8:(c + 1) * 8] @ cw[c].T          # (512, 7)
        hc = (g @ bmat.T)                                # (512, 26)
        for f in range(26):
            hmt[f * 64 + c, :] = hc[:, f]
    return hmt.astype(np.float32)


def _build_tbl(chan_b, mix_w):
    pos = np.arange(NT, dtype=np.float64)[:, None]
    half = HID // 2
    div = np.exp(np.arange(half, dtype=np.float32).astype(np.float64)
                 * (-np.log(10000.0) / half))
    ang = (pos * div[None, :]).astype(np.float32)
    pe = np.zeros((NT, HID), np.float32)
    pe[:, 0::2] = np.sin(ang)
    pe[:, 1::2] = np.cos(ang)
    const0 = (mix_w.astype(np.float64) @ chan_b.reshape(-1).astype(np.float64))
    tbl = np.zeros((2048, HID), np.float32)
    tbl[:NT] = pe + const0.astype(np.float32)[None, :]
    return tbl


def _pretranspose(sig):
    """(65536, 64) -> [128, (513, 64)]: x[p, q, c] = sig[128q + p, c], zero-padded
    final q col. Per-SBUF-partition rows become fully contiguous HBM runs."""
    out = np.zeros((128, Q + 1, C), np.float32)
    out[:, :Q, :] = sig.reshape(Q, 128, C).transpose(1, 0, 2)
    return np.ascontiguousarray(out.reshape(128, (Q + 1) * C))


# ------------------------------------------------------------- device program
_NC_CACHE = None


def _build_nc(repeat=1, stage=5):
    nc = bacc.Bacc("TRN2", target_bir_lowering=False, debug=False, num_devices=8)
    x = nc.dram_tensor("x", [128, (Q + 1) * C], F32R, kind="ExternalInput")
    dm = nc.dram_tensor("dm", [128, 320], F32R, kind="ExternalInput")
    hmt = nc.dram_tensor("hmt", [NKT * 128, HID], F32R, kind="ExternalInput")
    tbl = nc.dram_tensor("tbl", [2048, HID], F32, kind="ExternalInput")
    z = nc.dram_tensor("z", [2048, HID], F32, kind="ExternalOutput")

    xv = x.rearrange("p (q c) -> p q c", c=C)            # host pre-transposed
    hmv = hmt.rearrange("(k p) h -> p k h", p=128)
    tbv = tbl.rearrange("(a i g w) h -> a g i w h", i=32, g=2, w=4)
    zv = z.rearrange("(a i g w) h -> a g i w h", i=32, g=2, w=4)

    with tile.TileContext(nc) as tc:
        with (
            tc.tile_pool(name="consts", bufs=1) as consts,
            tc.tile_pool(name="xp", bufs=2) as xp,
            tc.tile_pool(name="magp", bufs=2) as magp,
            tc.tile_pool(name="fsp", bufs=2) as fsp,
            tc.tile_pool(name="sqp", bufs=4) as sqp,
            tc.tile_pool(name="sqpp", bufs=2) as sqpp,
            tc.tile_pool(name="sgp", bufs=2) as sgp,
            tc.tile_pool(name="mtp", bufs=2) as mtp,
            tc.tile_pool(name="tbp", bufs=2) as tbp,
            tc.tile_pool(name="zsp", bufs=2) as zsp,
            tc.tile_pool(name="psA", bufs=3, space="PSUM") as psA,
            tc.tile_pool(name="psC", bufs=2, space="PSUM") as psC,
        ):
            dm_sb = consts.tile([128, 320], F32R)
            nc.sync.dma_start(out=dm_sb, in_=dm[:, :])
            hm_sb = consts.tile([128, NKT, HID], F32R)
            nc.sync.dma_start(out=hm_sb, in_=hmv)

            import contextlib
            rep_ctx = tc.For_i(0, repeat, 1) if repeat > 1 else contextlib.nullcontext()
            with rep_ctx:
                _kernel_body(nc, tc, locals(), stage)
    nc.compile()
    return nc


def _kernel_body(nc, tc, env, stage=5):
    consts = env["consts"]; xp = env["xp"]; magp = env["magp"]
    fsp = env["fsp"]; sqp = env["sqp"]; sqpp = env["sqpp"]; mtp = env["mtp"]
    sgp = env["sgp"]
    tbp = env["tbp"]; zsp = env["zsp"]; psA = env["psA"]; psC = env["psC"]
    dm_sb = env["dm_sb"]; hm_sb = env["hm_sb"]
    xv = env["xv"]; tbv = env["tbv"]; zv = env["zv"]
    if True:
            for sc in range(NSC):
                # ---- load x superchunk (+1 q-col halo) -------------------
                x_sb = xp.tile([128, QSC + 1, C], F32R)
                nc.sync.dma_start(out=x_sb[:, :, :],
                                  in_=xv[:, QSC * sc: QSC * sc + QSC + 1, :])

                if stage < 2:
                    continue
                mag_sb = magp.tile([128, 24, NST, 4], F32R)   # (k, i, w)
                f0_sb = fsp.tile([128, NST * 4], F32R)
                sg_sb = fsp.tile([128, NST * 4], F32)
                std_sb = fsp.tile([128, NST * 4], F32R)

                for jb in range(NST // 4):        # 4 stationaries per psum tile
                    pA4 = psA.tile([128, 4, 256], F32, name="pA4")
                    for il in range(4):
                        i = jb * 4 + il
                        # stationary cols m = q2*64 + c  <- x-col 2i+q2
                        lhs1 = x_sb[:, 2 * i:2 * i + 2, :]
                        nc.tensor.matmul(pA4[:, il, :], lhs1,
                                         dm_sb[:, 0:256],
                                         start=True, stop=False)
                        lhs2 = x_sb[:, 2 * i + 1:2 * i + 3, :]
                        nc.tensor.matmul(pA4[:, il, 192:256], lhs2,
                                         dm_sb[:, 256:320],
                                         start=False, stop=True,
                                         skip_group_check=True)
                    if stage < 3:
                        continue
                    pv = pA4.rearrange("p s (w f) -> p s w f", w=4)
                    sq_sb = sqp.tile([128, 4, 4, 63], mybir.dt.bfloat16)
                    nc.scalar.activation(sq_sb, pv[:, :, :, 1:64], AF.Square)
                    if jb % 2 == 0:
                        sqp_sb = sqpp.tile([128, 8, 4, 31], mybir.dt.bfloat16,
                                           name="sqp_sb")
                    sh = sqp_sb[:, 4 * (jb % 2):4 * (jb % 2) + 4, :, :]
                    # re^2 + im^2: cos_k col k-1 (k=1..31), sin_k col 31+k
                    nc.vector.tensor_add(sh, sq_sb[:, :, :, 0:31],
                                         sq_sb[:, :, :, 32:63])
                    sl = slice(16 * jb, 16 * jb + 16)
                    sgr = sg_sb[:, sl].rearrange("p (s w) -> p s w", w=4)
                    # sigma partial: cos_32^2 (sq col 31); reduce adds pairs
                    nc.vector.tensor_copy(sgr, sq_sb[:, :, :, 31])
                    nc.vector.tensor_copy(f0_sb[:, sl], pv[:, :, :, 0])
                    if jb % 2 == 1:
                        sl8 = slice(16 * (jb - 1), 16 * (jb - 1) + 32)
                        sgr8 = sg_sb[:, sl8].rearrange("p (s w) -> p s w", w=4)
                        red8 = sgp.tile([128, 8, 4], F32, name="red8")
                        nc.vector.tensor_reduce(red8, sqp_sb,
                                                mybir.AxisListType.X,
                                                mybir.AluOpType.add)
                        nc.vector.tensor_add(sgr8, sgr8, red8)
                        magv = (mag_sb[:, :, 4 * (jb - 1):4 * (jb - 1) + 8, :]
                                .transpose([0, 2, 3, 1]))    # (p, s, w, k)
                        nc.scalar.activation(magv, sqp_sb[:, :, :, 0:24],
                                             AF.Sqrt)
                if stage >= 3:
                    nc.scalar.activation(std_sb[:, :], sg_sb[:, :], AF.Sqrt)

                # ---- B->C transpose DMAs --------------------------------
                if stage < 4:
                    continue
                mt_sb = mtp.tile([128, NKT, 256], F32R)
                # mag k-slot f = 2*kt + fh -> one DMA per (fh, q2) covers kt=0..11
                magr = mag_sb.rearrange("p (k2 two) i w -> p two k2 i w", two=2)
                for fh in range(2):
                    for q2 in range(2):
                        nc.sync.dma_start(
                            out=mt_sb[64 * fh:64 * fh + 64, 0:12,
                                      q2 * 128:(q2 + 1) * 128],
                            in_=magr[64 * q2:64 * q2 + 64, fh, :, :, :])
                        src = f0_sb if fh == 0 else std_sb
                        nc.sync.dma_start(
                            out=mt_sb[64 * fh:64 * fh + 64, 12,
                                      q2 * 128:(q2 + 1) * 128],
                            in_=src[64 * q2:64 * q2 + 64, :])

                # ---- phase C: z = MT.T @ HmT + tbl ----------------------
                if stage < 5:
                    continue
                for g in range(2):
                    zP = psC.tile([128, HID], F32)
                    for kt in range(NKT):
                        nc.tensor.matmul(
                            zP,
                            mt_sb[:, kt, g * 128:(g + 1) * 128],
                            hm_sb[:, kt, :],
                            start=(kt == 0), stop=(kt == NKT - 1))
                    tb_sb = tbp.tile([128, HID], F32)
                    nc.sync.dma_start(out=tb_sb, in_=tbv[sc, g])
                    zs = zsp.tile([128, HID], F32)
                    nc.vector.tensor_add(zs, zP, tb_sb)
                    nc.sync.dma_start(out=zv[sc, g], in_=zs)
            if stage < 5:
                zdummy = zsp.tile([128, HID], F32)
                nc.vector.memset(zdummy, 0.0)
                nc.sync.dma_start(out=env["z"][0:128, :], in_=zdummy)


def kernel(signal, chan_w, chan_b, mix_w, marker):
    global _NC_CACHE, LAST_RESULTS
    signal = np.ascontiguousarray(np.asarray(signal, np.float32))
    chan_w = np.asarray(chan_w, np.float32)
    chan_b = np.asarray(chan_b, np.float32)
    mix_w = np.asarray(mix_w, np.float32)
    marker = np.asarray(marker, np.float32)

    dmat = _build_d()
    hmt = np.ascontiguousarray(_build_hmt(chan_w, mix_w))
    tbl = np.ascontiguousarray(_build_tbl(chan_b, mix_w))

    r = _get_runner()
    per_core = [{"x": _pretranspose(signal[b]),
                 "dm": dmat, "hmt": hmt, "tbl": tbl} for b in range(B)]
    concat_in = [np.concatenate([per_core[c][n] for c in range(B)], axis=0)
                 for n in r["in_names"]]
    concat_zeros = [np.zeros((B * s[0],) + s[1:], d) for s, d in r["out_specs"]]
    out_arrs = r["fn"](*concat_in, *concat_zeros)
    z_all = np.asarray(out_arrs[0]).reshape(B, 2048, HID)

    out = np.empty((B, 2048, HID), np.float32)
    for b in range(B):
        out[b, 0] = marker
        out[b, 1:] = z_all[b, :NT]
    return out


def bench(signal, chan_w, chan_b, mix_w, marker, iters=10):
    """Time device execution with inputs already resident on device."""
    import time
    r = _get_runner()
    signal = np.ascontiguousarray(np.asarray(signal, np.float32))
    dmat = _build_d()
    hmt = np.ascontiguousarray(_build_hmt(np.asarray(chan_w, np.float32),
                                          np.asarray(mix_w, np.float32)))
    tbl = np.ascontiguousarray(_build_tbl(np.asarray(chan_b, np.float32),
                                          np.asarray(mix_w, np.float32)))
    per_core = [{"x": _pretranspose(signal[b]),
                 "dm": dmat, "hmt": hmt, "tbl": tbl} for b in range(B)]
    concat_in = [np.concatenate([per_core[c][n] for c in range(B)], axis=0)
                 for n in r["in_names"]]
    shard = NamedSharding(r["mesh"], PartitionSpec("core"))
    dev_in = [jax.device_put(a, shard) for a in concat_in]
    jax.block_until_ready(dev_in)
    times = []
    for _ in range(iters):
        concat_zeros = [jax.device_put(np.zeros((B * s[0],) + s[1:], d), shard)
                        for s, d in r["out_specs"]]
        jax.block_until_ready(concat_zeros)
        t0 = time.perf_counter()
        out = r["fn"](*dev_in, *concat_zeros)
        jax.block_until_ready(out)
        times.append(time.perf_counter() - t0)
    return times


_RUNNER = None


def _get_runner():
    global _RUNNER
    if _RUNNER is not None:
        return _RUNNER
    nc = _build_nc()
    bass2jax.install_neuronx_cc_hook()
    pname = nc.partition_id_tensor.name if nc.partition_id_tensor else None
    in_names, out_names, out_avals = [], [], []
    for alloc in nc.m.functions[0].allocations:
        if not isinstance(alloc, mybir.MemoryLocationSet):
            continue
        name = alloc.memorylocations[0].name
        if alloc.kind == "ExternalInput":
            if name != pname:
                in_names.append(name)
        elif alloc.kind == "ExternalOutput":
            out_names.append(name)
            out_avals.append(jax.core.ShapedArray(
                tuple(alloc.tensor_shape), mybir.dt.np(alloc.dtype)))
    n_params, n_outs = len(in_names), len(out_names)
    all_in = in_names + out_names
    if pname is not None:
        all_in = all_in + [pname]

    def _body(*args):
        operands = list(args)
        if pname is not None:
            operands.append(bass2jax.partition_id_tensor())
        outs = bass2jax._bass_exec_p.bind(
            *operands, out_avals=tuple(out_avals), in_names=tuple(all_in),
            out_names=tuple(out_names), lowering_input_output_aliases=(),
            sim_require_finite=True, sim_require_nnan=True, nc=nc)
        return tuple(outs)

    devices = jax.devices()[:B]
    mesh = Mesh(np.asarray(devices), ("core",))
    fn = jax.jit(
        shard_map(_body, mesh=mesh,
                  in_specs=(PartitionSpec("core"),) * (n_params + n_outs),
                  out_specs=(PartitionSpec("core"),) * n_outs,
                  check_rep=False),
        donate_argnums=tuple(range(n_params, n_params + n_outs)),
        keep_unused=True)
    _RUNNER = {"fn": fn, "in_names": in_names, "out_names": out_names,
               "out_specs": [(tuple(a.shape), a.dtype) for a in out_avals],
               "mesh": mesh}
    return _RUNNER


if __name__ == "__main__":
    rng = np.random.default_rng(0)
    o = kernel(
        signal=rng.standard_normal((B, T, C), dtype=np.float32),
        chan_w=0.02 * rng.standard_normal((C, 7, 8)).astype(np.float32),
        chan_b=0.02 * rng.standard_normal((C, 8)).astype(np.float32),
        mix_w=0.02 * rng.standard_normal((HID, HID)).astype(np.float32),
        marker=0.02 * rng.standard_normal((HID,)).astype(np.float32),
    )
    print(o.shape, o.dtype)
